# revision 40
# baseline (speedup 1.0000x reference)
"""Trainium2 Bass kernel for nn_Attention_60833916781258 (GAT-style complex attention).

Reference computation (B=2, N=4096, F=128, U=64):
    X_re = H_re @ W ; X_im = H_im @ W
    s = X @ a_1 ; n = X @ a_2 (per re/im)
    E = leaky_relu(s_i + n_j, 0.2)
    alpha1 = softmax(E_re + NEG_BIG*(1-A)) ; alpha2 = softmax(E_im)
    out_re = alpha1 @ X_re - alpha2 @ X_im ; out_im = alpha1 @ X_im + alpha2 @ X_re

Sharding: 8 cores; core c handles batch b=c//4, query-row block rb=c%4 (1024 rows).

Math trick: exp(lrelu(t)) = e^{0.2t} * max(e^{0.8t}, 1).  The per-row factor
e^{0.2 s_i} cancels in the row softmax, so the unnormalized weights are
    u[j,i] = A[i,j] * max(e^{0.8 s_i + n_j}, e^{0.2 n_j}) / 16
(1/16 keeps fp16 row-sum accumulators below 65504; it cancels too).

The joint exponential is SEPARABLE: e^{0.8 s_i + n_j} = e^{0.8 s_i} * e^{n_j}.
Precompute once: sbe = e^{0.8 s_i} broadcast over partitions ([128,1024],
reused every chunk) plus per-key scalars fexp_j = e^{n_j - ln16},
floor_j = e^{0.2 n_j}/16.  Per 128-key chunk:
    m1 = (sbe_re * fexp_k) max floor_k        -- ONE 4x-mode DVE tensor_scalar
    u  = m1 * A_tile                          -- ONE 2x-mode DVE tensor_tensor
    v  = (sbe_im * fexp_im_k) max floor_im_k  -- ONE 4x-mode DVE tensor_scalar
No scalar-engine work in the main loop.  Row sums for BOTH u and v are
interleaved ones-matmuls on the PE, all four sum rows sharing ONE psum bank
at base partitions 0/32 (su) and 64/96 (sv).

The whole kernel is pipelined in 4 groups of 8 key chunks: each group's ht
DMA piece, X/s/n setup matmuls, fexp/floor/xcat preps and main-loop chunks
are issued per group, so the PE starts multiplying while later groups' input
DMA and setup still run.  ht pieces are separate tiles spread over 4 DGE
queues so tile-level dependencies stay per-group.

Layout is key-major [j, i] so the alpha@X contraction runs on the tensor
engine without transposing attention tiles.  The epilogue transposes the
numerators on the PE in fp16 (4 row blocks per PSUM bank) and combines
straight from PSUM with work spread over Act (wu = ru*tu), DVE and Pool
(scalar_tensor_tensor adds of +-rv*tv), writing one [1024, 128] fp16
output (re | im) per core.

Note: dma_start_transpose (XBAR) looked attractive for the transposes, but
InstDmaTransposeAnt is invisible to the tile framework's dependency tracking
(get_accessed_tiles returns []), so it races with producers/consumers and
intermittently corrupts results.  Keep transposes on the PE.
"""

import sys

if "/opt/trn_rl_repo" not in sys.path:
    sys.path.insert(0, "/opt/trn_rl_repo")

import math

import numpy as np

import concourse.bass as bass
import concourse.tile as tile
from concourse import bacc, mybir
from concourse.bass_utils import run_bass_kernel_spmd

B, N, F, U = 2, 4096, 128, 64
NCORES = 8
ROWS = N * B // NCORES  # 1024 query rows per core
NCHUNK = N // 128  # 32 key chunks of 128
GS = [7, 7, 7, 7, 4]  # chunks per pipeline group (7*66 cols fit one psum bank)
KOFF = [0, 7, 14, 21, 28]
NGRP = len(GS)
LN16 = math.log(16.0)
A_ = mybir.AluOpType
AF = mybir.ActivationFunctionType
f32 = mybir.dt.float32
f16 = mybir.dt.float16

_PROGRAM_CACHE = {}


def _build_program():
    if "nc" in _PROGRAM_CACHE:
        return _PROGRAM_CACHE["nc"]

    nc = bacc.Bacc("TRN2", target_bir_lowering=False, debug=False, num_devices=NCORES)
    from concourse.tile_rust import add_dep_helper

    _pe_prev = [None]

    def mm(out, lhsT, rhs, reuse=False, **kw):
        bi = nc.tensor.matmul(out, lhsT=lhsT, rhs=rhs, **kw)
        if reuse:
            bi.ins.ldweights = False
        if _pe_prev[0] is not None:
            add_dep_helper(bi.ins, _pe_prev[0], sync=False, reason="pe order")
        _pe_prev[0] = bi.ins
        return bi

    dp = nc.dram_tensor
    ht_in = {
        "re": dp("ht_re", [F, N], f16, kind="ExternalInput").ap(),
        "im": dp("ht_im", [F, N], f16, kind="ExternalInput").ap(),
    }
    wsn_in = dp("wsn", [F, 65], f16, kind="ExternalInput").ap()
    wa8_in = dp("wa8", [F, 1], f32, kind="ExternalInput").ap()
    amul_in = dp("amul", [N, ROWS], f16, kind="ExternalInput").ap()
    ident_in = dp("ident", [128, 128], f32, kind="ExternalInput").ap()
    identh_in = dp("identh", [128, 128], f16, kind="ExternalInput").ap()
    o_cat = dp("ocat", [128, 8 * 128], f16, kind="ExternalOutput").ap()

    with tile.TileContext(nc) as tc:
        with tc.tile_pool(name="cst", bufs=1) as cst:
            # ---- constants (wsn = [W | W@a1 | W@a2], wa8 = 0.8*W@a1, host-side)
            # DMA queue plan: the critical-path tensors lead their queues.
            #   sync:   ht_re piece0, then am chunks (even)
            #   scalar: ht_im piece0, idents, then am chunks (odd)
            #   gpsimd: wa8, wsn, ht pieces 1-3, output at the end
            wsn_sb = cst.tile([F, 65], f16, tag="wsn", name="wsn")
            wa8_sb = cst.tile([F, 1], f32, tag="wa8", name="wa8")
            nc.gpsimd.dma_start(wa8_sb[:], wa8_in[:])
            nc.gpsimd.dma_start(wsn_sb[:], wsn_in[:])
            ones16_sb = cst.tile([128, 1], f16, tag="ones16", name="ones16")
            nc.gpsimd.memset(ones16_sb[:], 1.0)
            ones128_sb = cst.tile([128, 128], f16, tag="ones128", name="ones128")
            nc.gpsimd.memset(ones128_sb[:], 1.0)
            negln16_sb = cst.tile([128, 1], f32, tag="negln16", name="negln16")
            nc.gpsimd.memset(negln16_sb[:], -LN16)

            # ---- ht in per-group piece tiles, spread over 4 DGE queues
            # ht: 2 DMAs per tensor (group-0 piece first, rest in one big DMA)
            # to minimize per-DMA completion-semaphore latency (~2.2us each).
            ht_sb = {}
            _defer_b = []
            for nm, q in (("re", nc.sync), ("im", nc.scalar)):
                ta = cst.tile([F, 1024], f16, tag=f"htA_{nm}", name=f"htA_{nm}")
                tb = cst.tile([F, 3072], f16, tag=f"htB_{nm}", name=f"htB_{nm}")
                q.dma_start(ta[:], ht_in[nm][:, 0:1024])
                # B piece triggered after group 0's am prefetch (queue order)
                _defer_b.append((q, tb, ht_in[nm][:, 1024:4096]))
                ht_sb[nm] = (ta, tb)

            def ht_chunk(nm, k):  # [128, 128] ht slice for key chunk k
                ta, tb = ht_sb[nm]
                if k < 8:
                    return ta[:, 128 * k : 128 * (k + 1)]
                return tb[:, 128 * (k - 8) : 128 * (k - 7)]

            # identity matrices (needed only in the epilogue; DMA triggered
            # late, after the loop's am DMAs are queued)
            ident_sb = cst.tile([128, 128], f32, tag="ident", name="ident")
            identh_sb = cst.tile([128, 128], f16, tag="identh", name="identh")

            # w18[f,p] = 0.8*(W@a1)[f] for all p
            w18_sb = cst.tile([128, 128], f16, tag="w18", name="w18")
            nc.vector.tensor_scalar(
                w18_sb[:], ones128_sb[:], wa8_sb[:, 0:1], None, op0=A_.mult
            )

            sbc8_sb = {
                nm: cst.tile([128, ROWS], f16, tag=f"sbc8_{nm}", name=f"sbc8_{nm}")
                for nm in ("re", "im")
            }
            sbe_sb = {
                nm: cst.tile([128, ROWS], f16, tag=f"sbe_{nm}", name=f"sbe_{nm}")
                for nm in ("re", "im")
            }

            with (
                tc.tile_pool(name="psX", bufs=2, space="PSUM") as psX,
                tc.tile_pool(name="psM", bufs=1, space="PSUM") as psM,
                tc.tile_pool(name="xsn", bufs=2) as xsn_pool,
                tc.tile_pool(name="ffl", bufs=2) as ffl_pool,
                tc.tile_pool(name="amp", bufs=2) as am_pool,
                tc.tile_pool(name="uvp", bufs=8) as uv_pool,
            ):
                # main-loop psum: numerators + row-sum banks (h at base
                # partitions 0/32 per the PE base-partition rule)
                psum_u = [
                    psM.tile([128, 512], f32, tag=f"pu{h}", name=f"pu{h}")
                    for h in range(2)
                ]
                psum_v = [
                    psM.tile([128, 512], f32, tag=f"pv{h}", name=f"pv{h}")
                    for h in range(2)
                ]
                psum_su = psM.tile([128, 512], f32, tag="psu", name="psu")
                psum_sv = psM.tile([128, 512], f32, tag="psv", name="psv")

                # sbc8: s for own rows (key cols 0:1024 = group 0 pieces),
                # broadcast to all partitions, scaled by 0.8.  One-shot
                # matmuls borrowing the numerator psum banks (drained before
                # the loop's start=True re-initializes them).
                for ni, nm in enumerate(("re", "im")):
                    for h in range(2):
                        sb_ps = (psum_u, psum_v)[ni][h]
                        mm(
                            sb_ps[:],
                            w18_sb[:],
                            ht_sb[nm][0][:, 512 * h : 512 * (h + 1)],
                            start=True,
                            stop=True,
                        )
                        if h == 0:
                            nc.scalar.copy(sbc8_sb[nm][:, 0:512], sb_ps[:])
                        else:
                            nc.vector.tensor_copy(sbc8_sb[nm][:, 512:1024], sb_ps[:])
                    if nm == "re":
                        # sbe = e^{0.8 s} broadcast tile; the im-side exp is
                        # issued AFTER group 0's fexp/floor so the Act queue
                        # doesn't delay the first m1
                        nc.scalar.activation(sbe_sb[nm][:], sbc8_sb[nm][:], AF.Exp)

                b3 = lambda t: t[:].rearrange("p (k o) -> p k o", o=1)

                def setup_group(g):
                    gc = GS[g]
                    k0 = KOFF[g]
                    # ---- group setup: X/s/n key-major via 66-col matmuls
                    # xsn_g = [re chunks | im chunks], each chunk
                    # [key, 0:64=X | 64=s | 65=n].  The numerator-matmul
                    # weights are read straight out of this tile via a
                    # 2-piece strided AP (see wview) -- no xcat assembly.
                    # per-chunk layout: [X_re(64) | X_im(64) | n_re | n_im]
                    # so the weights view [X_re|X_im] is ONE contiguous AP
                    xsn_g = xsn_pool.tile(
                        [128, 7 * 130], f16, tag="xsn", name=f"xsn{g}"
                    )
                    xsn5 = xsn_g[:].rearrange("p (c w) -> p c w", w=130)
                    # prefetch this group's adjacency in ONE DMA
                    amg = am_pool.tile([128, 7 * ROWS], f16, tag="am", name="am")
                    amq = nc.sync if g % 2 == 0 else nc.scalar
                    am3 = amg[:].rearrange("p (c r) -> p c r", c=7)
                    am_src = amul_in[:].rearrange("(c p) r -> p c r", p=128)
                    if g == 0:
                        # chunk 0 in its own DMA so the first u-mult isn't
                        # gated by the whole group's transfer
                        amq.dma_start(am3[:, 0:1, :], am_src[:, 0:1, :])
                        amq.dma_start(am3[:, 1:gc, :], am_src[:, 1:gc, :])
                    else:
                        amq.dma_start(am3[:, 0:gc, :], am_src[:, k0 : k0 + gc, :])
                    if g == 0:
                        for q, tb, ap in _defer_b:
                            q.dma_start(tb[:], ap)
                    if g == 1:
                        nc.scalar.dma_start(ident_sb[:], ident_in[:])
                        nc.scalar.dma_start(identh_sb[:], identh_in[:])
                    # PE: all X/s/n matmuls back-to-back, one psum tile per
                    # tensor (no intra-group WAR with psX bufs=2)
                    xgs = []
                    for ti, nm in enumerate(("re", "im")):
                        xg = psX.tile([128, 455], f32, tag="xg", name="xg")
                        xg3 = xg[:].rearrange("p (c u) -> p c u", u=65)
                        xgs.append(xg3)
                        for m in range(gc):
                            mm(
                                xg3[:, m, :],
                                ht_chunk(nm, k0 + m),
                                wsn_sb[:],
                                start=True,
                                stop=True,
                            )
                        # n col drains on DVE right away (feeds fexp)
                        nc.vector.tensor_copy(
                            xsn5[:, 0:gc, 128 + ti : 129 + ti],
                            xg3[:, 0:gc, 64:65],
                        )
                    # per-key scalars first (gate the loop's DVE ops) ...
                    fexp_g = ffl_pool.tile([128, 2 * 7], f32, tag="fx", name=f"fx{g}")
                    floor_g = ffl_pool.tile([128, 2 * 7], f32, tag="fl", name=f"fl{g}")
                    for ti in range(2):
                        n_ap = xsn5[:, 0:gc, 128 + ti : 129 + ti]
                        nc.scalar.activation(
                            b3(fexp_g[:, 7 * ti : 7 * ti + gc]),
                            n_ap,
                            AF.Exp,
                            bias=negln16_sb[:],
                            scale=1.0,
                        )
                        nc.scalar.activation(
                            b3(floor_g[:, 7 * ti : 7 * ti + gc]),
                            n_ap,
                            AF.Exp,
                            bias=negln16_sb[:],
                            scale=0.2,
                        )
                    # ... then the X columns (gate the loop's PE matmuls)
                    for ti, xg3 in enumerate(xgs):
                        # group 0: DVE (Act is busy with the sbe exps and
                        # would gate the loop start); later groups: Act
                        # (DVE is the loop's busiest engine)
                        if g == 0:
                            nc.vector.tensor_copy(
                                xsn5[:, 0:gc, 64 * ti : 64 * ti + 64],
                                xg3[:, 0:gc, 0:64],
                            )
                        else:
                            nc.scalar.copy(
                                xsn5[:, 0:gc, 64 * ti : 64 * ti + 64],
                                xg3[:, 0:gc, 0:64],
                            )
                    # weights: [X_re_kk | X_im_kk], one contiguous 128-col AP
                    wview = lambda kk: xsn5[:, kk, 0:128]
                    return wview, fexp_g, floor_g, am3

                def loop_chunk(g, kk, gctx):
                    wview, fexp_g, floor_g, am3 = gctx
                    if True:
                        k = KOFF[g] + kk
                        am_t = am3[:, kk, :]
                        # m1 = (sbe_re * fexp_k) max floor_k  -- 4x-mode DVE
                        m1 = uv_pool.tile([128, ROWS], f16, tag="m1", name="m1")
                        nc.vector.tensor_scalar(
                            m1[:],
                            sbe_sb["re"][:],
                            fexp_g[:, kk : kk + 1],
                            floor_g[:, kk : kk + 1],
                            op0=A_.mult,
                            op1=A_.max,
                        )
                        u_t = uv_pool.tile([128, ROWS], f16, tag="u", name="u")
                        nc.vector.tensor_tensor(u_t[:], m1[:], am_t, op=A_.mult)
                        v_t = uv_pool.tile([128, ROWS], f16, tag="v", name="v")
                        nc.vector.tensor_scalar(
                            v_t[:],
                            sbe_sb["im"][:],
                            fexp_g[:, GC + kk : GC + kk + 1],
                            floor_g[:, GC + kk : GC + kk + 1],
                            op0=A_.mult,
                            op1=A_.max,
                        )

                        st, sp = (k == 0), (k == NCHUNK - 1)
                        for h in range(2):
                            mm(
                                psum_u[h][:],
                                wview(kk),
                                u_t[:, 512 * h : 512 * (h + 1)],
                                start=st,
                                stop=sp,
                                reuse=(h != 0),
                            )
                        for h in range(2):
                            mm(
                                psum_v[h][:],
                                wview(kk),
                                v_t[:, 512 * h : 512 * (h + 1)],
                                start=st,
                                stop=sp,
                                reuse=True,
                            )
                        for h in range(2):
                            mm(
                                psum_su[32 * h : 32 * h + 1, :],
                                ones16_sb[:],
                                u_t[:, 512 * h : 512 * (h + 1)],
                                start=st,
                                stop=sp,
                                reuse=(not st) or (h != 0),
                            )
                        for h in range(2):
                            mm(
                                psum_sv[32 * h : 32 * h + 1, :],
                                ones16_sb[:],
                                v_t[:, 512 * h : 512 * (h + 1)],
                                start=st,
                                stop=sp,
                                reuse=True,
                            )

                # ---- pipelined driver: issue group g+1's setup mid-way
                # through group g's chunks so its Act/DVE prep chain hides
                # under the PE-bound loop instead of stalling it.
                gctx = setup_group(0)
                nc.scalar.activation(sbe_sb["im"][:], sbc8_sb["im"][:], AF.Exp)
                for g in range(NGRP):
                    nxt = None
                    for kk in range(GC):
                        if kk == 2 and g + 1 < NGRP:
                            nxt = setup_group(g + 1)
                        loop_chunk(g, kk, gctx)
                    if nxt is not None:
                        gctx = nxt

                # ---- drains: numerators to fp16 SBUF, row sums to suv
                with tc.tile_pool(name="fin", bufs=1) as fin:
                    cu_sb = fin.tile([128, ROWS], f16, tag="cu", name="cu")
                    cv_sb = fin.tile([128, ROWS], f16, tag="cv", name="cv")
                    for h in range(2):
                        nc.scalar.copy(cu_sb[:, 512 * h : 512 * (h + 1)], psum_u[h][:])
                        nc.vector.tensor_copy(
                            cv_sb[:, 512 * h : 512 * (h + 1)], psum_v[h][:]
                        )
                    su_sb = fin.tile([128, 512], f32, tag="su", name="su")
                    sv_sb = fin.tile([128, 512], f32, tag="sv", name="sv")
                    nc.scalar.copy(su_sb[:], psum_su[:])
                    nc.vector.tensor_copy(sv_sb[:], psum_sv[:])

            # ---- epilogue
            with (
                tc.tile_pool(name="psE", bufs=1, space="PSUM") as psE,
                tc.tile_pool(name="ep2", bufs=1) as ep2,
            ):
                # row-sum transposes: [1, 128] block -> one rsT column each;
                # row-block it = 4h + j lives at su/sv[32h, 128j:128j+128]
                rsT_ps = psE.tile([128, 16], f32, tag="rsT", name="rsT")
                for it in range(8):
                    h, j = divmod(it, 4)
                    for si, src in enumerate((su_sb, sv_sb)):
                        p = 32 * h
                        mm(
                            rsT_ps[:, 8 * si + it : 8 * si + it + 1],
                            src[p : p + 1, 128 * j : 128 * (j + 1)],
                            ident_sb[p : p + 1, p : p + 1],
                            is_transpose=True,
                            start=True,
                            stop=True,
                        )
                rr_sb = ep2.tile([128, 16], f32, tag="rr", name="rr")
                nc.vector.reciprocal(rr_sb[:], rsT_ps[:])
                # negated reciprocals for the subtract-as-add trick
                nrr_sb = ep2.tile([128, 16], f32, tag="nrr", name="nrr")
                nc.vector.tensor_scalar(
                    nrr_sb[:], rr_sb[:], -1.0, None, op0=A_.mult
                )

                # numerator transposes on the PE (fp16, 4 blocks per bank);
                # the combine reads straight from PSUM
                tu_ps = [
                    psE.tile([128, 512], f16, tag=f"tu{b}", name=f"tu{b}")
                    for b in range(2)
                ]
                tv_ps = [
                    psE.tile([128, 512], f16, tag=f"tv{b}", name=f"tv{b}")
                    for b in range(2)
                ]
                ocat_sb = ep2.tile([128, 8 * 128], f16, tag="ocat", name="ocat")
                ocat3 = ocat_sb[:].rearrange("p (c u) -> p c u", u=128)
                wu_sb = ep2.tile([128, 8 * 128], f16, tag="wu", name="wu")
                wu3 = wu_sb[:].rearrange("p (c u) -> p c u", u=128)

                def combine(it):
                    b, j = divmod(it, 4)
                    ru = rr_sb[:, it : it + 1]
                    rv = rr_sb[:, 8 + it : 8 + it + 1]
                    nrv = nrr_sb[:, 8 + it : 8 + it + 1]
                    tub = tu_ps[b][:, 128 * j : 128 * (j + 1)]
                    tvb = tv_ps[b][:, 128 * j : 128 * (j + 1)]
                    # out_re = ru*tu_re - rv*tv_im ; out_im = ru*tu_im + rv*tv_re
                    wu = wu3[:, it, :]
                    nc.scalar.activation(wu, tub, AF.Copy, scale=ru)
                    e0 = e1 = nc.vector  # GpSimd cannot read PSUM
                    e0.scalar_tensor_tensor(
                        ocat3[:, it, 0:64],
                        tvb[:, 64:128],
                        nrv,
                        wu[:, 0:64],
                        A_.mult,
                        A_.add,
                    )
                    e1.scalar_tensor_tensor(
                        ocat3[:, it, 64:128],
                        tvb[:, 0:64],
                        rv,
                        wu[:, 64:128],
                        A_.mult,
                        A_.add,
                    )

                for bnk in range(2):
                    for it in range(4 * bnk, 4 * bnk + 4):
                        sl = slice(128 * it, 128 * (it + 1))
                        dl = slice(128 * (it % 4), 128 * (it % 4) + 128)
                        mm(
                            tu_ps[bnk][:, dl],
                            cu_sb[:, sl],
                            identh_sb[:],
                            is_transpose=True,
                            start=True,
                            stop=True,
                        )
                        mm(
                            tv_ps[bnk][:, dl],
                            cv_sb[:, sl],
                            identh_sb[:],
                            is_transpose=True,
                            start=True,
                            stop=True,
                        )
                    for it in range(4 * bnk, 4 * bnk + 4):
                        combine(it)
                    # partition-major DRAM layout: contiguous 1KB per
                    # partition per half -> fat DMA packets (host unscrambles)
                    nc.gpsimd.dma_start(
                        o_cat[:, 512 * bnk : 512 * (bnk + 1)],
                        ocat_sb[:, 512 * bnk : 512 * (bnk + 1)],
                    )

    nc.compile()
    _PROGRAM_CACHE["nc"] = nc
    return nc


# ---------------------------------------------------------------- host wrapper


def _make_in_maps(H_re, H_im, A, W, a_1, a_2):
    W32 = np.asarray(W, np.float32)
    acat = np.concatenate(
        [np.asarray(a_1, np.float32), np.asarray(a_2, np.float32)], axis=1
    )
    wa = W32 @ acat  # [F, 2] = [W@a1 | W@a2]
    wsn = np.concatenate([W32, wa[:, 1:2]], axis=1).astype(np.float16)  # [F, 65]
    wa8 = (0.8 * wa[:, 0:1]).astype(np.float32)
    shared = {
        "wsn": wsn,
        "wa8": wa8,
        "ident": np.eye(128, dtype=np.float32),
        "identh": np.eye(128, dtype=np.float16),
    }
    in_maps = []
    for c in range(NCORES):
        b, rb = divmod(c, NCORES // B)
        r0 = rb * ROWS
        hre = np.asarray(H_re[b], np.float32)
        him = np.asarray(H_im[b], np.float32)
        ab = np.asarray(A[b], np.float32)
        # key order rolled so this core's own query rows come first
        amul = np.ascontiguousarray(
            np.roll(ab[r0 : r0 + ROWS].T, -r0, axis=0).astype(np.float16)
        )
        in_maps.append(
            {
                **shared,
                "ht_re": np.ascontiguousarray(
                    np.roll(hre, -r0, axis=0).T.astype(np.float16)
                ),
                "ht_im": np.ascontiguousarray(
                    np.roll(him, -r0, axis=0).T.astype(np.float16)
                ),
                "amul": amul,
            }
        )
    return in_maps


def kernel(H_re, H_im, A, W, a_1, a_2):
    nc = _build_program()
    in_maps = _make_in_maps(H_re, H_im, A, W, a_1, a_2)
    res = run_bass_kernel_spmd(nc, in_maps, list(range(NCORES)))
    out_re = np.empty((B, N, U), np.float32)
    out_im = np.empty((B, N, U), np.float32)
    for c in range(NCORES):
        b, rb = divmod(c, NCORES // B)
        r0 = rb * ROWS
        oc = res.results[c]["ocat"].astype(np.float32)
        # device layout: [partition p, row-block c, channel] -> row 128c+p
        oc = oc.reshape(128, 8, 128).transpose(1, 0, 2).reshape(ROWS, 128)
        out_re[b, r0 : r0 + ROWS] = oc[:, 0:64]
        out_im[b, r0 : r0 + ROWS] = oc[:, 64:128]
    return out_re, out_im


# revision 42
# speedup vs baseline: 1.1820x; 1.1820x over previous
"""Trainium2 Bass kernel for nn_Attention_60833916781258 (GAT-style complex attention).

Reference computation (B=2, N=4096, F=128, U=64):
    X_re = H_re @ W ; X_im = H_im @ W
    s = X @ a_1 ; n = X @ a_2 (per re/im)
    E = leaky_relu(s_i + n_j, 0.2)
    alpha1 = softmax(E_re + NEG_BIG*(1-A)) ; alpha2 = softmax(E_im)
    out_re = alpha1 @ X_re - alpha2 @ X_im ; out_im = alpha1 @ X_im + alpha2 @ X_re

Sharding: 8 cores; core c handles batch b=c//4, query-row block rb=c%4 (1024 rows).

Math trick: exp(lrelu(t)) = e^{0.2t} * max(e^{0.8t}, 1).  The per-row factor
e^{0.2 s_i} cancels in the row softmax, so the unnormalized weights are
    u[j,i] = A[i,j] * max(e^{0.8 s_i + n_j}, e^{0.2 n_j}) / 16
(1/16 keeps fp16 row-sum accumulators below 65504; it cancels too).

The joint exponential is SEPARABLE: e^{0.8 s_i + n_j} = e^{0.8 s_i} * e^{n_j}.
Precompute once: sbe = e^{0.8 s_i} broadcast over partitions ([128,1024],
reused every chunk) plus per-key scalars fexp_j = e^{n_j - ln16},
floor_j = e^{0.2 n_j}/16.  Per 128-key chunk:
    m1 = (sbe_re * fexp_k) max floor_k        -- ONE 4x-mode DVE tensor_scalar
    u  = m1 * A_tile                          -- ONE 2x-mode DVE tensor_tensor
    v  = (sbe_im * fexp_im_k) max floor_im_k  -- ONE 4x-mode DVE tensor_scalar
No scalar-engine work in the main loop.  Row sums for BOTH u and v are
interleaved ones-matmuls on the PE, all four sum rows sharing ONE psum bank
at base partitions 0/32 (su) and 64/96 (sv).

The whole kernel is pipelined in 4 groups of 8 key chunks: each group's ht
DMA piece, X/s/n setup matmuls, fexp/floor/xcat preps and main-loop chunks
are issued per group, so the PE starts multiplying while later groups' input
DMA and setup still run.  ht pieces are separate tiles spread over 4 DGE
queues so tile-level dependencies stay per-group.

Layout is key-major [j, i] so the alpha@X contraction runs on the tensor
engine without transposing attention tiles.  The epilogue transposes the
numerators on the PE in fp16 (4 row blocks per PSUM bank) and combines
straight from PSUM with work spread over Act (wu = ru*tu), DVE and Pool
(scalar_tensor_tensor adds of +-rv*tv), writing one [1024, 128] fp16
output (re | im) per core.

Note: dma_start_transpose (XBAR) looked attractive for the transposes, but
InstDmaTransposeAnt is invisible to the tile framework's dependency tracking
(get_accessed_tiles returns []), so it races with producers/consumers and
intermittently corrupts results.  Keep transposes on the PE.
"""

import sys

if "/opt/trn_rl_repo" not in sys.path:
    sys.path.insert(0, "/opt/trn_rl_repo")

import math

import numpy as np

import concourse.bass as bass
import concourse.tile as tile
from concourse import bacc, mybir
from concourse.bass_utils import run_bass_kernel_spmd

B, N, F, U = 2, 4096, 128, 64
NCORES = 8
ROWS = N * B // NCORES  # 1024 query rows per core
NCHUNK = N // 128  # 32 key chunks of 128
GS = [7, 7, 7, 7, 4]  # chunks per pipeline group (7*66 cols fit one psum bank)
KOFF = [0, 7, 14, 21, 28]
NGRP = len(GS)
LN16 = math.log(16.0)
A_ = mybir.AluOpType
AF = mybir.ActivationFunctionType
f32 = mybir.dt.float32
f16 = mybir.dt.float16

_PROGRAM_CACHE = {}


def _build_program():
    if "nc" in _PROGRAM_CACHE:
        return _PROGRAM_CACHE["nc"]

    nc = bacc.Bacc("TRN2", target_bir_lowering=False, debug=False, num_devices=NCORES)
    from concourse.tile_rust import add_dep_helper

    _pe_prev = [None]

    def mm(out, lhsT, rhs, reuse=False, **kw):
        bi = nc.tensor.matmul(out, lhsT=lhsT, rhs=rhs, **kw)
        if reuse:
            bi.ins.ldweights = False
        if _pe_prev[0] is not None:
            add_dep_helper(bi.ins, _pe_prev[0], sync=False, reason="pe order")
        _pe_prev[0] = bi.ins
        return bi

    dp = nc.dram_tensor
    ht_in = {
        "re": dp("ht_re", [F, N], f16, kind="ExternalInput").ap(),
        "im": dp("ht_im", [F, N], f16, kind="ExternalInput").ap(),
    }
    wsn_in = dp("wsn", [F, 65], f16, kind="ExternalInput").ap()
    wa8_in = dp("wa8", [F, 1], f32, kind="ExternalInput").ap()
    amul_in = dp("amul", [N, ROWS], f16, kind="ExternalInput").ap()
    ident_in = dp("ident", [128, 128], f32, kind="ExternalInput").ap()
    identh_in = dp("identh", [128, 128], f16, kind="ExternalInput").ap()
    o_cat = dp("ocat", [128, 8 * 128], f16, kind="ExternalOutput").ap()

    with tile.TileContext(nc) as tc:
        with tc.tile_pool(name="cst", bufs=1) as cst:
            # ---- constants (wsn = [W | W@a1 | W@a2], wa8 = 0.8*W@a1, host-side)
            # DMA queue plan: the critical-path tensors lead their queues.
            #   sync:   ht_re piece0, then am chunks (even)
            #   scalar: ht_im piece0, idents, then am chunks (odd)
            #   gpsimd: wa8, wsn, ht pieces 1-3, output at the end
            wsn_sb = cst.tile([F, 65], f16, tag="wsn", name="wsn")
            wa8_sb = cst.tile([F, 1], f32, tag="wa8", name="wa8")
            nc.gpsimd.dma_start(wa8_sb[:], wa8_in[:])
            nc.gpsimd.dma_start(wsn_sb[:], wsn_in[:])
            ones16_sb = cst.tile([128, 1], f16, tag="ones16", name="ones16")
            nc.gpsimd.memset(ones16_sb[:], 1.0)
            ones128_sb = cst.tile([128, 128], f16, tag="ones128", name="ones128")
            nc.gpsimd.memset(ones128_sb[:], 1.0)
            negln16_sb = cst.tile([128, 1], f32, tag="negln16", name="negln16")
            nc.gpsimd.memset(negln16_sb[:], -LN16)

            # ---- ht in per-group piece tiles, spread over 4 DGE queues
            # ht: 2 DMAs per tensor (group-0 piece first, rest in one big DMA)
            # to minimize per-DMA completion-semaphore latency (~2.2us each).
            ht_sb = {}
            _defer_b = []
            for nm, q in (("re", nc.sync), ("im", nc.scalar)):
                ta = cst.tile([F, 1024], f16, tag=f"htA_{nm}", name=f"htA_{nm}")
                tb = cst.tile([F, 3072], f16, tag=f"htB_{nm}", name=f"htB_{nm}")
                q.dma_start(ta[:], ht_in[nm][:, 0:1024])
                # B piece triggered after group 0's am prefetch (queue order)
                _defer_b.append((q, tb, ht_in[nm][:, 1024:4096]))
                ht_sb[nm] = (ta, tb)

            def ht_chunk(nm, k):  # [128, 128] ht slice for key chunk k
                ta, tb = ht_sb[nm]
                if k < 8:
                    return ta[:, 128 * k : 128 * (k + 1)]
                return tb[:, 128 * (k - 8) : 128 * (k - 7)]

            # identity matrices (needed only in the epilogue; DMA triggered
            # late, after the loop's am DMAs are queued)
            ident_sb = cst.tile([128, 128], f32, tag="ident", name="ident")
            identh_sb = cst.tile([128, 128], f16, tag="identh", name="identh")

            # w18[f,p] = 0.8*(W@a1)[f] for all p
            w18_sb = cst.tile([128, 128], f16, tag="w18", name="w18")
            nc.vector.tensor_scalar(
                w18_sb[:], ones128_sb[:], wa8_sb[:, 0:1], None, op0=A_.mult
            )

            sbc8_sb = {
                nm: cst.tile([128, ROWS], f16, tag=f"sbc8_{nm}", name=f"sbc8_{nm}")
                for nm in ("re", "im")
            }
            sbe_sb = {
                nm: cst.tile([128, ROWS], f16, tag=f"sbe_{nm}", name=f"sbe_{nm}")
                for nm in ("re", "im")
            }

            with (
                tc.tile_pool(name="psX", bufs=2, space="PSUM") as psX,
                tc.tile_pool(name="psM", bufs=1, space="PSUM") as psM,
                tc.tile_pool(name="xsn", bufs=2) as xsn_pool,
                tc.tile_pool(name="ffl", bufs=2) as ffl_pool,
                tc.tile_pool(name="amp", bufs=2) as am_pool,
                tc.tile_pool(name="uvp", bufs=8) as uv_pool,
            ):
                # main-loop psum: numerators + row-sum banks (h at base
                # partitions 0/32 per the PE base-partition rule)
                psum_u = [
                    psM.tile([128, 512], f32, tag=f"pu{h}", name=f"pu{h}")
                    for h in range(2)
                ]
                psum_v = [
                    psM.tile([128, 512], f32, tag=f"pv{h}", name=f"pv{h}")
                    for h in range(2)
                ]
                psum_su = psM.tile([128, 512], f32, tag="psu", name="psu")
                psum_sv = psM.tile([128, 512], f32, tag="psv", name="psv")

                # sbc8: s for own rows (key cols 0:1024 = group 0 pieces),
                # broadcast to all partitions, scaled by 0.8.  One-shot
                # matmuls borrowing the numerator psum banks (drained before
                # the loop's start=True re-initializes them).
                for ni, nm in enumerate(("re", "im")):
                    for h in range(2):
                        sb_ps = (psum_u, psum_v)[ni][h]
                        mm(
                            sb_ps[:],
                            w18_sb[:],
                            ht_sb[nm][0][:, 512 * h : 512 * (h + 1)],
                            start=True,
                            stop=True,
                        )
                        if h == 0:
                            nc.scalar.copy(sbc8_sb[nm][:, 0:512], sb_ps[:])
                        else:
                            nc.vector.tensor_copy(sbc8_sb[nm][:, 512:1024], sb_ps[:])
                    if nm == "re":
                        # sbe = e^{0.8 s} broadcast tile; the im-side exp is
                        # issued AFTER group 0's fexp/floor so the Act queue
                        # doesn't delay the first m1
                        nc.scalar.activation(sbe_sb[nm][:], sbc8_sb[nm][:], AF.Exp)

                b3 = lambda t: t[:].rearrange("p (k o) -> p k o", o=1)

                def setup_group(g):
                    gc = GS[g]
                    k0 = KOFF[g]
                    # ---- group setup: X/s/n key-major via 66-col matmuls
                    # xsn_g = [re chunks | im chunks], each chunk
                    # [key, 0:64=X | 64=s | 65=n].  The numerator-matmul
                    # weights are read straight out of this tile via a
                    # 2-piece strided AP (see wview) -- no xcat assembly.
                    # per-chunk layout: [X_re(64) | X_im(64) | n_re | n_im]
                    # so the weights view [X_re|X_im] is ONE contiguous AP
                    xsn_g = xsn_pool.tile(
                        [128, 7 * 130], f16, tag="xsn", name=f"xsn{g}"
                    )
                    xsn5 = xsn_g[:].rearrange("p (c w) -> p c w", w=130)
                    # prefetch this group's adjacency in ONE DMA
                    amg = am_pool.tile([128, 7 * ROWS], f16, tag="am", name="am")
                    amq = nc.sync if g % 2 == 0 else nc.scalar
                    am3 = amg[:].rearrange("p (c r) -> p c r", c=7)
                    am_src = amul_in[:].rearrange("(c p) r -> p c r", p=128)
                    if g == 0:
                        # chunk 0 in its own DMA so the first u-mult isn't
                        # gated by the whole group's transfer
                        amq.dma_start(am3[:, 0:1, :], am_src[:, 0:1, :])
                        amq.dma_start(am3[:, 1:gc, :], am_src[:, 1:gc, :])
                    else:
                        amq.dma_start(am3[:, 0:gc, :], am_src[:, k0 : k0 + gc, :])
                    if g == 0:
                        for q, tb, ap in _defer_b:
                            q.dma_start(tb[:], ap)
                    if g == 1:
                        nc.scalar.dma_start(ident_sb[:], ident_in[:])
                        nc.scalar.dma_start(identh_sb[:], identh_in[:])
                    # PE: all X/s/n matmuls back-to-back, one psum tile per
                    # tensor (no intra-group WAR with psX bufs=2)
                    xgs = []
                    for ti, nm in enumerate(("re", "im")):
                        xg = psX.tile([128, 455], f32, tag="xg", name="xg")
                        xg3 = xg[:].rearrange("p (c u) -> p c u", u=65)
                        xgs.append(xg3)
                        for m in range(gc):
                            mm(
                                xg3[:, m, :],
                                ht_chunk(nm, k0 + m),
                                wsn_sb[:],
                                start=True,
                                stop=True,
                            )
                        # n col drains on DVE right away (feeds fexp)
                        nc.vector.tensor_copy(
                            xsn5[:, 0:gc, 128 + ti : 129 + ti],
                            xg3[:, 0:gc, 64:65],
                        )
                    # per-key scalars first (gate the loop's DVE ops) ...
                    fexp_g = ffl_pool.tile([128, 2 * 7], f32, tag="fx", name=f"fx{g}")
                    floor_g = ffl_pool.tile([128, 2 * 7], f32, tag="fl", name=f"fl{g}")
                    for ti in range(2):
                        n_ap = xsn5[:, 0:gc, 128 + ti : 129 + ti]
                        nc.scalar.activation(
                            b3(fexp_g[:, 7 * ti : 7 * ti + gc]),
                            n_ap,
                            AF.Exp,
                            bias=negln16_sb[:],
                            scale=1.0,
                        )
                        nc.scalar.activation(
                            b3(floor_g[:, 7 * ti : 7 * ti + gc]),
                            n_ap,
                            AF.Exp,
                            bias=negln16_sb[:],
                            scale=0.2,
                        )
                    # ... then the X columns (gate the loop's PE matmuls)
                    for ti, xg3 in enumerate(xgs):
                        # group 0: DVE (Act is busy with the sbe exps and
                        # would gate the loop start); later groups: Act
                        # (DVE is the loop's busiest engine)
                        if g == 0:
                            nc.vector.tensor_copy(
                                xsn5[:, 0:gc, 64 * ti : 64 * ti + 64],
                                xg3[:, 0:gc, 0:64],
                            )
                        else:
                            nc.scalar.copy(
                                xsn5[:, 0:gc, 64 * ti : 64 * ti + 64],
                                xg3[:, 0:gc, 0:64],
                            )
                    # weights: [X_re_kk | X_im_kk], one contiguous 128-col AP
                    wview = lambda kk: xsn5[:, kk, 0:128]
                    return wview, fexp_g, floor_g, am3

                def loop_chunk(g, kk, gctx):
                    wview, fexp_g, floor_g, am3 = gctx
                    if True:
                        k = KOFF[g] + kk
                        am_t = am3[:, kk, :]
                        # m1 = (sbe_re * fexp_k) max floor_k  -- 4x-mode DVE
                        m1 = uv_pool.tile([128, ROWS], f16, tag="m1", name="m1")
                        nc.vector.tensor_scalar(
                            m1[:],
                            sbe_sb["re"][:],
                            fexp_g[:, kk : kk + 1],
                            floor_g[:, kk : kk + 1],
                            op0=A_.mult,
                            op1=A_.max,
                        )
                        u_t = uv_pool.tile([128, ROWS], f16, tag="u", name="u")
                        nc.vector.tensor_tensor(u_t[:], m1[:], am_t, op=A_.mult)
                        v_t = uv_pool.tile([128, ROWS], f16, tag="v", name="v")
                        nc.vector.tensor_scalar(
                            v_t[:],
                            sbe_sb["im"][:],
                            fexp_g[:, GC + kk : GC + kk + 1],
                            floor_g[:, GC + kk : GC + kk + 1],
                            op0=A_.mult,
                            op1=A_.max,
                        )

                        st, sp = (k == 0), (k == NCHUNK - 1)
                        for h in range(2):
                            mm(
                                psum_u[h][:],
                                wview(kk),
                                u_t[:, 512 * h : 512 * (h + 1)],
                                start=st,
                                stop=sp,
                                reuse=(h != 0),
                            )
                        for h in range(2):
                            mm(
                                psum_v[h][:],
                                wview(kk),
                                v_t[:, 512 * h : 512 * (h + 1)],
                                start=st,
                                stop=sp,
                                reuse=True,
                            )
                        for h in range(2):
                            mm(
                                psum_su[32 * h : 32 * h + 1, :],
                                ones16_sb[:],
                                u_t[:, 512 * h : 512 * (h + 1)],
                                start=st,
                                stop=sp,
                                reuse=(not st) or (h != 0),
                            )
                        for h in range(2):
                            mm(
                                psum_sv[32 * h : 32 * h + 1, :],
                                ones16_sb[:],
                                v_t[:, 512 * h : 512 * (h + 1)],
                                start=st,
                                stop=sp,
                                reuse=True,
                            )

                # ---- pipelined driver: issue group g+1's setup mid-way
                # through group g's chunks so its Act/DVE prep chain hides
                # under the PE-bound loop instead of stalling it.
                gctx = setup_group(0)
                nc.scalar.activation(sbe_sb["im"][:], sbc8_sb["im"][:], AF.Exp)
                for g in range(NGRP):
                    nxt = None
                    for kk in range(GC):
                        if kk == 2 and g + 1 < NGRP:
                            nxt = setup_group(g + 1)
                        loop_chunk(g, kk, gctx)
                    if nxt is not None:
                        gctx = nxt

                # ---- drains: numerators to fp16 SBUF, row sums to suv
                with tc.tile_pool(name="fin", bufs=1) as fin:
                    cu_sb = fin.tile([128, ROWS], f16, tag="cu", name="cu")
                    cv_sb = fin.tile([128, ROWS], f16, tag="cv", name="cv")
                    for h in range(2):
                        nc.scalar.copy(cu_sb[:, 512 * h : 512 * (h + 1)], psum_u[h][:])
                        nc.vector.tensor_copy(
                            cv_sb[:, 512 * h : 512 * (h + 1)], psum_v[h][:]
                        )
                    su_sb = fin.tile([128, 512], f32, tag="su", name="su")
                    sv_sb = fin.tile([128, 512], f32, tag="sv", name="sv")
                    nc.scalar.copy(su_sb[:], psum_su[:])
                    nc.vector.tensor_copy(sv_sb[:], psum_sv[:])

            # ---- epilogue
            with (
                tc.tile_pool(name="psE", bufs=1, space="PSUM") as psE,
                tc.tile_pool(name="ep2", bufs=1) as ep2,
            ):
                # row-sum transposes: [1, 128] block -> one rsT column each;
                # row-block it = 4h + j lives at su/sv[32h, 128j:128j+128]
                rsT_ps = psE.tile([128, 16], f32, tag="rsT", name="rsT")
                for it in range(8):
                    h, j = divmod(it, 4)
                    for si, src in enumerate((su_sb, sv_sb)):
                        p = 32 * h
                        mm(
                            rsT_ps[:, 8 * si + it : 8 * si + it + 1],
                            src[p : p + 1, 128 * j : 128 * (j + 1)],
                            ident_sb[p : p + 1, p : p + 1],
                            is_transpose=True,
                            start=True,
                            stop=True,
                        )
                rr_sb = ep2.tile([128, 16], f32, tag="rr", name="rr")
                nc.vector.reciprocal(rr_sb[:], rsT_ps[:])
                # negated reciprocals for the subtract-as-add trick
                nrr_sb = ep2.tile([128, 16], f32, tag="nrr", name="nrr")
                nc.vector.tensor_scalar(
                    nrr_sb[:], rr_sb[:], -1.0, None, op0=A_.mult
                )

                # numerator transposes on the PE (fp16, 4 blocks per bank);
                # the combine reads straight from PSUM
                tu_ps = [
                    psE.tile([128, 512], f16, tag=f"tu{b}", name=f"tu{b}")
                    for b in range(2)
                ]
                tv_ps = [
                    psE.tile([128, 512], f16, tag=f"tv{b}", name=f"tv{b}")
                    for b in range(2)
                ]
                ocat_sb = ep2.tile([128, 8 * 128], f16, tag="ocat", name="ocat")
                ocat3 = ocat_sb[:].rearrange("p (c u) -> p c u", u=128)
                wu_sb = ep2.tile([128, 8 * 128], f16, tag="wu", name="wu")
                wu3 = wu_sb[:].rearrange("p (c u) -> p c u", u=128)

                def combine(it):
                    b, j = divmod(it, 4)
                    ru = rr_sb[:, it : it + 1]
                    rv = rr_sb[:, 8 + it : 8 + it + 1]
                    nrv = nrr_sb[:, 8 + it : 8 + it + 1]
                    tub = tu_ps[b][:, 128 * j : 128 * (j + 1)]
                    tvb = tv_ps[b][:, 128 * j : 128 * (j + 1)]
                    # out_re = ru*tu_re - rv*tv_im ; out_im = ru*tu_im + rv*tv_re
                    wu = wu3[:, it, :]
                    nc.scalar.activation(wu, tub, AF.Copy, scale=ru)
                    e0 = e1 = nc.vector  # GpSimd cannot read PSUM
                    e0.scalar_tensor_tensor(
                        ocat3[:, it, 0:64],
                        tvb[:, 64:128],
                        nrv,
                        wu[:, 0:64],
                        A_.mult,
                        A_.add,
                    )
                    e1.scalar_tensor_tensor(
                        ocat3[:, it, 64:128],
                        tvb[:, 0:64],
                        rv,
                        wu[:, 64:128],
                        A_.mult,
                        A_.add,
                    )

                for bnk in range(2):
                    for it in range(4 * bnk, 4 * bnk + 4):
                        sl = slice(128 * it, 128 * (it + 1))
                        dl = slice(128 * (it % 4), 128 * (it % 4) + 128)
                        mm(
                            tu_ps[bnk][:, dl],
                            cu_sb[:, sl],
                            identh_sb[:],
                            is_transpose=True,
                            start=True,
                            stop=True,
                        )
                        mm(
                            tv_ps[bnk][:, dl],
                            cv_sb[:, sl],
                            identh_sb[:],
                            is_transpose=True,
                            start=True,
                            stop=True,
                        )
                    for it in range(4 * bnk, 4 * bnk + 4):
                        combine(it)
                    # partition-major DRAM layout: contiguous 1KB per
                    # partition per half -> fat DMA packets (host unscrambles)
                    nc.gpsimd.dma_start(
                        o_cat[:, 512 * bnk : 512 * (bnk + 1)],
                        ocat_sb[:, 512 * bnk : 512 * (bnk + 1)],
                    )

    nc.compile()
    _PROGRAM_CACHE["nc"] = nc
    return nc


# ---------------------------------------------------------------- host wrapper


def _make_in_maps(H_re, H_im, A, W, a_1, a_2):
    W32 = np.asarray(W, np.float32)
    acat = np.concatenate(
        [np.asarray(a_1, np.float32), np.asarray(a_2, np.float32)], axis=1
    )
    wa = W32 @ acat  # [F, 2] = [W@a1 | W@a2]
    wsn = np.concatenate([W32, wa[:, 1:2]], axis=1).astype(np.float16)  # [F, 65]
    wa8 = (0.8 * wa[:, 0:1]).astype(np.float32)
    shared = {
        "wsn": wsn,
        "wa8": wa8,
        "ident": np.eye(128, dtype=np.float32),
        "identh": np.eye(128, dtype=np.float16),
    }
    in_maps = []
    for c in range(NCORES):
        b, rb = divmod(c, NCORES // B)
        r0 = rb * ROWS
        hre = np.asarray(H_re[b], np.float32)
        him = np.asarray(H_im[b], np.float32)
        ab = np.asarray(A[b], np.float32)
        # key order rolled so this core's own query rows come first
        amul = np.ascontiguousarray(
            np.roll(ab[r0 : r0 + ROWS].T, -r0, axis=0).astype(np.float16)
        )
        in_maps.append(
            {
                **shared,
                "ht_re": np.ascontiguousarray(
                    np.roll(hre, -r0, axis=0).T.astype(np.float16)
                ),
                "ht_im": np.ascontiguousarray(
                    np.roll(him, -r0, axis=0).T.astype(np.float16)
                ),
                "amul": amul,
            }
        )
    return in_maps


def kernel(H_re, H_im, A, W, a_1, a_2):
    nc = _build_program()
    in_maps = _make_in_maps(H_re, H_im, A, W, a_1, a_2)
    res = run_bass_kernel_spmd(nc, in_maps, list(range(NCORES)))
    out_re = np.empty((B, N, U), np.float32)
    out_im = np.empty((B, N, U), np.float32)
    for c in range(NCORES):
        b, rb = divmod(c, NCORES // B)
        r0 = rb * ROWS
        oc = res.results[c]["ocat"].astype(np.float32)
        # device layout: [partition p, row-block c, channel] -> row 128c+p
        oc = oc.reshape(128, 8, 128).transpose(1, 0, 2).reshape(ROWS, 128)
        out_re[b, r0 : r0 + ROWS] = oc[:, 0:64]
        out_im[b, r0 : r0 + ROWS] = oc[:, 64:128]
    return out_re, out_im
